# revision 1
# baseline (speedup 1.0000x reference)
"""Trainium2 Bass kernel for nn_BertSelfAttention_ling (relative_key_query
position embeddings + char/word level biases).

Sharding: pure data-parallel over batch - 16 batches / 8 cores = 2 per core,
no collectives. Weights/embeddings replicated.

v2 design (vs the DRAM-scratch baseline):
  - host preps x^T (bf16) so no on-device X transposes, and the char/word
    level bias fully looked-up AND pre-sheared into k-window coordinates
    (biasW), so the device does no Horner polynomial and no bias transposes.
  - q/k relative-position windows [128, 639] per (head, tile) stay in SBUF;
    the diagonal "shear" gather runs as an SBUF->SBUF DMA with a custom
    affine AP (partition stride = row_pitch - 1) - no DRAM round trip.
  - k-window evictions ADD biasW (tensor_tensor) so the keg gather directly
    yields keg+bias; one identity matmul accumulates it into scores.
  - head pairs (even head at partitions 0-63, odd at 64-127) have their
    64-contraction matmuls (QK^T, windows) emitted interleaved so they can
    run concurrently on distinct PE row-groups.
  - softmax: exp on ACT reading PSUM, attention_mask as per-partition bias;
    1/sqrt(D) folded into Wq / et8 / bias tables on host.
  - PV: ctxT = V_aug^T @ probsT with a ones column producing the softmax
    denominator; bf16 PE transpose back to [l, d]; normalize at ctx.
"""

import numpy as np

_CACHE0 = {}


def _enable_ldw_opt():
    """Let walrus merge/dedupe LDWEIGHTS (off by default in this harness)."""
    if _CACHE0.get("patched"):
        return
    _CACHE0["patched"] = True
    try:
        from concourse import bass_utils as _bu
        _orig = _bu.run_command

        def _patched(cmd, *a, **kw):
            cmd = [c.replace("--enable-ldw-opt=false", "--enable-ldw-opt=true")
                   if isinstance(c, str) else c for c in cmd]
            return _orig(cmd, *a, **kw)

        _bu.run_command = _patched
    except Exception:
        pass


B, S, H, D = 16, 512, 12, 64
HID = H * D
P = 128
NLT = S // P            # 4 tiles of 128 along S
W = P + S - 1           # 639 window width
BPC = B // 8            # batches per core = 2
FREE = NLT * W          # 2556 window elems per partition per (head, term)

_CACHE = {}


def _build_program(repeat=1, use_dmat=None):
    import os
    if os.environ.get("KERNEL_LDW_OPT", "0") == "1":
        _enable_ldw_opt()
    if use_dmat is None:
        use_dmat = os.environ.get("KERNEL_USE_DMAT", "0") == "1"
    import concourse.bass as bass
    import concourse.bacc as bacc
    import concourse.mybir as mybir
    from concourse.tile import TileContext
    from concourse.masks import make_identity

    dt = mybir.dt
    AF = mybir.ActivationFunctionType
    OP = mybir.AluOpType

    nc = bacc.Bacc(None, target_bir_lowering=False, debug=False)

    # ---------------- external tensors ----------------
    xt_in = nc.dram_tensor("xt", [BPC, HID, S], dt.bfloat16,
                            kind="ExternalInput")
    bw_in = nc.dram_tensor("biasw", [BPC, NLT, P, W], dt.bfloat16,
                           kind="ExternalInput")
    mask_in = nc.dram_tensor("maskr", [BPC, S], dt.float32, kind="ExternalInput")
    wall_in = nc.dram_tensor("wall", [18, P, HID], dt.bfloat16,
                             kind="ExternalInput")
    bqk_in = nc.dram_tensor("bqk", [12, P], dt.float32, kind="ExternalInput")
    bv_in = nc.dram_tensor("bv2", [1, HID], dt.float32, kind="ExternalInput")
    etr_in = nc.dram_tensor("etr2", [P, 2 * S - 1], dt.bfloat16,
                            kind="ExternalInput")
    et8_in = nc.dram_tensor("et82", [P, 2 * S - 1], dt.bfloat16,
                            kind="ExternalInput")
    out_dram = nc.dram_tensor("out", [BPC, H, D + 1, S], dt.bfloat16,
                              kind="ExternalOutput")

    with TileContext(nc) as tc:
        with (
            tc.tile_pool(name="const", bufs=1) as constp,
            tc.tile_pool(name="persist", bufs=1) as persist,
            tc.tile_pool(name="win_sb", bufs=3) as win_sb,
            tc.tile_pool(name="qeg_sb", bufs=3) as qeg_sb,
            tc.tile_pool(name="qt_sb", bufs=2) as qt_sb,
            tc.tile_pool(name="keg_sb", bufs=3) as keg_sb,
            tc.tile_pool(name="probs_sb", bufs=3) as probs_sb,
            tc.tile_pool(name="misc_sb", bufs=2) as misc_sb,
            # PSUM: 8 banks: winA 2 + winB 2 + sc 2 + ctxT 2
            tc.tile_pool(name="winA_ps", bufs=1, space="PSUM") as winA_ps,
            tc.tile_pool(name="winB_ps", bufs=1, space="PSUM") as winB_ps,
            tc.tile_pool(name="sc_ps", bufs=2, space="PSUM") as sc_ps,
            tc.tile_pool(name="ctxT_ps", bufs=2, space="PSUM") as ctxT_ps,
        ):
            # ------------ constants ------------
            ident = constp.tile([P, P], dt.bfloat16)
            make_identity(nc, ident[:])

            etr = constp.tile([P, 2 * S - 1], dt.bfloat16)   # two stacked copies
            nc.sync.dma_start(etr[:], etr_in[:, :])
            et8 = constp.tile([P, 2 * S - 1], dt.bfloat16)
            nc.sync.dma_start(et8[:], et8_in[:, :])

            # per-partition bias layouts: bqk_sb[p, 0:6]=bq/8, [p, 6:12]=bk
            bqk_sb = constp.tile([P, 12], dt.float32)
            nc.sync.dma_start(bqk_sb[:], bqk_in[:, :].rearrange("c p -> p c"))
            bq_sb = bqk_sb[:, 0:6]
            bk_sb = bqk_sb[:, 6:12]
            bv_sb = constp.tile([P, HID], dt.float32)
            nc.sync.dma_start(bv_sb[:], bv_in[:, :].to_broadcast([P, HID]))

            mask_sb = constp.tile([P, BPC * NLT], dt.float32)
            nc.sync.dma_start(
                mask_sb[:],
                mask_in[:, :].rearrange("b (rt p) -> p (b rt)", p=P))

            # weights packed [18, 128, 768]: wq tiles 0-5, wk 6-11, wv 12-17
            wq_sb = persist.tile([P, 6 * HID], dt.bfloat16)
            wk_sb = persist.tile([P, 6 * HID], dt.bfloat16)
            wv_sb = persist.tile([P, 6 * HID], dt.bfloat16)
            nc.sync.dma_start(
                wq_sb[:].rearrange("p (t c) -> p t c", t=6),
                wall_in[0:6].rearrange("t p c -> p t c"))
            nc.sync.dma_start(
                wk_sb[:].rearrange("p (t c) -> p t c", t=6),
                wall_in[6:12].rearrange("t p c -> p t c"))
            nc.sync.dma_start(
                wv_sb[:].rearrange("p (t c) -> p t c", t=6),
                wall_in[12:18].rearrange("t p c -> p t c"))

            for _rep in range(repeat):  # repeat>1: timing variant
                # ------------ load x^T and sheared bias ------------
                xt = {}
                bw = {}
                for b in range(BPC):
                    xt[b] = persist.tile([P, 6 * S], dt.bfloat16, tag=f"xt{b}", name=f"xt{b}")
                    nc.scalar.dma_start(
                        xt[b][:].rearrange("p (t s) -> p t s", t=6),
                        xt_in[b].rearrange("(t p) s -> p t s", p=P))
                    bw[b] = persist.tile([P, NLT * W], dt.bfloat16, tag=f"bw{b}", name=f"bw{b}")
                    nc.scalar.dma_start(
                        bw[b][:].rearrange("p (t w) -> p t w", w=W),
                        bw_in[b].rearrange("t p w -> p t w"))

                # ------------ Q/K projections (LDW shared across batches) ----
                qt = {}
                kt = {}
                for b in range(BPC):
                    qt[b] = persist.tile([P, 6 * S], dt.bfloat16, tag=f"qt{b}", name=f"qt{b}")
                    kt[b] = persist.tile([P, 6 * S], dt.bfloat16, tag=f"kt{b}", name=f"kt{b}")
                def emit_proj(ot):
                    for wsb, boff, scl, dst in ((wq_sb, 0, 1.0 / 64, qt),
                                                (wk_sb, 6, 1.0 / 8, kt)):
                        ps = {}
                        for b in range(BPC):
                            ps[b] = sc_ps.tile([P, S], dt.float32, tag="sc", name=f"projps{b}")
                        for it in range(6):
                            lhsT = wsb[:, it * HID + ot * P:
                                       it * HID + (ot + 1) * P]
                            for b in range(BPC):
                                nc.tensor.matmul(
                                    ps[b][:], lhsT,
                                    xt[b][:, it * S:(it + 1) * S],
                                    start=(it == 0), stop=(it == 5))
                        for b in range(BPC):
                            bias_ap = bqk_sb[:, boff + ot: boff + ot + 1]
                            nc.vector.scalar_tensor_tensor(
                                dst[b][:, ot * S:(ot + 1) * S], ps[b][:],
                                scl, bias_ap.to_broadcast([P, S]),
                                op0=OP.mult, op1=OP.add)

                # ------------ V projection (x^T tiles stationary) ------------
                # emitted AFTER the windows prefill so the first shear
                # gathers overlap the V matmuls during pipeline fill
                vaug = {}

                def emit_v(b):
                    vaug[b] = persist.tile([P, NLT * (H * 65)], dt.bfloat16,
                                           tag=f"va{b}", name=f"va{b}")
                    for rt in range(NLT):
                        base = rt * (H * 65)
                        pso = {}
                        for oc in range(2):
                            pso[oc] = ctxT_ps.tile([P, 384], dt.float32,
                                                   tag="ctxT", name=f"vps{oc}")
                        for it in range(6):
                            lhsT = xt[b][:, it * S + rt * P: it * S + (rt + 1) * P]
                            for oc in range(2):
                                nc.tensor.matmul(
                                    pso[oc][:], lhsT,
                                    wv_sb[:, it * HID + oc * 384:
                                          it * HID + (oc + 1) * 384],
                                    start=(it == 0), stop=(it == 5))
                        for oc in range(2):
                            dst = vaug[b][:, base + oc * 6 * 65:
                                          base + (oc + 1) * 6 * 65] \
                                .rearrange("p (h c) -> p h c", c=65)[:, :, 0:64]
                            bvb = bv_sb[:, oc * 384:(oc + 1) * 384] \
                                .rearrange("p (h c) -> p h c", c=64)
                            nc.vector.scalar_tensor_tensor(
                                dst, pso[oc][:].rearrange("p (h c) -> p h c", c=64),
                                0.125, bvb, op0=OP.mult, op1=OP.add)
                        ones = vaug[b][:, base: base + H * 65] \
                            .rearrange("p (h c) -> p h c", c=65)[:, :, 64:65]
                        nc.gpsimd.memset(ones, 1.0)

                # ------------ attention: 12 head-pairs (2 per batch step) ----
                # software pipeline: emit windows+gathers for pair p+1 before
                # phase B of pair p so the gathers overlap phase-B PE work.
                ESPLIT = 294  # DVE/ACT balanced split point for evictions

                def windows(b, hp):
                    """window matmuls + evictions + shear gathers for the
                    head pair (2*hp, 2*hp+1) of batch b. Returns qeg/keg."""
                    res = {}
                    for di, (src, ee) in ((1, (kt[b], et8)), (0, (qt[b], etr))):
                        wins = {}
                        for hh, (po, wps) in enumerate(((0, winA_ps), (64, winB_ps))):
                            wins[hh] = win_sb.tile([P, FREE], dt.bfloat16,
                                                   tag=f"win{hh}", name=f"win{hh}")
                        for t in range(NLT):
                            lo = 384 - P * t
                            pss = {}
                            for hh, (po, wps) in enumerate(
                                    ((0, winA_ps), (64, winB_ps))):
                                ps = wps.tile([P, W], dt.float32, tag="w", name=f"wps{hh}")
                                pss[hh] = ps
                                lhsT = src[po:po + D,
                                           hp * S + t * P: hp * S + (t + 1) * P]
                                nc.tensor.matmul(ps[:, 0:S], lhsT,
                                                 ee[po:po + D, lo:lo + S],
                                                 start=True, stop=True)
                            for hh, po in ((0, 0), (1, 64)):
                                lhsT = src[po:po + D,
                                           hp * S + t * P: hp * S + (t + 1) * P]
                                nc.tensor.matmul(pss[hh][:, S:W], lhsT,
                                                 ee[po:po + D, lo + S:lo + W],
                                                 start=True, stop=True)
                            for hh in range(2):
                                dst = wins[hh][:, t * W:(t + 1) * W]
                                ps = pss[hh]
                                if di == 0:  # q-term copy: alternate ACT/DVE
                                    if hh == 0:
                                        nc.scalar.activation(dst, ps[:],
                                                             AF.Copy)
                                    else:
                                        nc.vector.tensor_copy(dst, ps[:])
                                else:  # k-term: add pre-sheared bias on DVE
                                    bwt = bw[b][:, t * W:(t + 1) * W]
                                    nc.vector.tensor_tensor(
                                        dst, ps[:], bwt, op=OP.add)
                        for hh in range(2):
                            src_ap = wins[hh][:]
                            if di == 0:
                                # q-term: shear gather [l', (lt, r)]; optional
                                # xbar transpose into QT[r', lt*512+rt*128+l']
                                g = qeg_sb.tile([P, NLT * S], dt.bfloat16,
                                                tag=f"g{hh}", name=f"g{hh}")
                                diag = bass.AP(
                                    src_ap.tensor, src_ap.offset + 127,
                                    [[FREE - 1, P], [W, NLT], [1, S]])
                                nc.sync.dma_start(
                                    g[:].rearrange("p (t j) -> p t j", j=S),
                                    diag)
                                if use_dmat:
                                    QT = qt_sb.tile([P, NLT * S], dt.bfloat16,
                                                    tag=f"qt{hh}", name=f"qtg{hh}")
                                    out_ap = bass.AP(
                                        QT[:].tensor, QT[:].offset,
                                        [[NLT * S, P], [P, 16], [1, P]])
                                    nc.sync.dma_start(out_ap, g[:],
                                                      transpose=True)
                                    res[(0, hh)] = QT
                                else:
                                    res[(0, hh)] = g
                            else:
                                # k-term (incl. bias): plain shear gather,
                                # dest [p, (rt, l)]
                                kg = keg_sb.tile([P, NLT * S], dt.bfloat16,
                                                 tag=f"k{hh}", name=f"k{hh}")
                                diag = bass.AP(
                                    src_ap.tensor, src_ap.offset + 127,
                                    [[FREE - 1, P], [W, NLT], [1, S]])
                                nc.sync.dma_start(
                                    kg[:].rearrange("p (t j) -> p t j", j=S),
                                    diag)
                                res[(1, hh)] = kg
                    return res

                def phase_b(b, hp, g):
                    """scores/softmax/PV for head pair (2*hp, 2*hp+1)."""
                    ctxT = {}
                    for hh in range(2):
                        ctxT[hh] = ctxT_ps.tile([D + 1, S], dt.float32,
                                                tag="ctxT", name=f"ctxT{hh}")
                    sc = {}
                    probs = {}
                    phist = {}
                    def emit_pv(rt):
                        for hh in range(2):
                            h = 2 * hp + hh
                            nc.tensor.matmul(
                                ctxT[hh][:],
                                vaug[b][:, rt * H * 65 + h * 65:
                                        rt * H * 65 + (h + 1) * 65],
                                phist[rt][hh][:], start=(rt == 0),
                                stop=(rt == 3))
                    for rt in range(NLT):
                        for hh, po in ((0, 0), (1, 64)):
                            sc[hh] = sc_ps.tile([P, S], dt.float32, tag="sc", name=f"sc{hh}")
                            nc.tensor.matmul(
                                sc[hh][:],
                                kt[b][po:po + D,
                                      hp * S + rt * P: hp * S + (rt + 1) * P],
                                qt[b][po:po + D, hp * S:(hp + 1) * S],
                                start=True, stop=False, skip_group_check=True)
                        for hh in range(2):
                            nc.tensor.matmul(
                                sc[hh][:], ident[:],
                                g[(1, hh)][:, rt * S:(rt + 1) * S],
                                start=False, stop=False, skip_group_check=True)
                        for hh in range(2):
                            qeg = g[(0, hh)]
                            for lt in range(NLT):
                                nc.tensor.matmul(
                                    sc[hh][:, lt * P:(lt + 1) * P],
                                    qeg[:, lt * S + rt * P:
                                        lt * S + (rt + 1) * P],
                                    ident[:], start=False,
                                    stop=(lt == NLT - 1),
                                    skip_group_check=True)
                        for hh in range(2):
                            pr = probs_sb.tile([P, S], dt.bfloat16, tag=f"pr{hh}", name=f"pr{hh}")
                            probs[hh] = pr
                            nc.scalar.activation(
                                pr[:], sc[hh][:], AF.Exp,
                                bias=mask_sb[:, b * NLT + rt: b * NLT + rt + 1],
                                scale=1.0)
                        phist[rt] = dict(probs)
                        if rt > 0:
                            emit_pv(rt - 1)
                    emit_pv(NLT - 1)

                    # ctxT evict (bf16) then store; host transposes+normalizes
                    for hh in range(2):
                        h = 2 * hp + hh
                        ctxT_sb = misc_sb.tile([D + 1, S], dt.bfloat16,
                                               tag=f"ctxTsb{hh}")
                        nc.vector.tensor_copy(ctxT_sb[:], ctxT[hh][:])
                        nc.sync.dma_start(out_dram[b, h], ctxT_sb[:])

                # pair schedule with 1-deep windows lookahead
                pairs = [(b, hp) for b in range(BPC) for hp in range(6)]
                gmap = {}
                DEPTH = 3
                for ot in range(6):
                    emit_proj(ot)
                for j in range(DEPTH):
                    gmap[pairs[j]] = windows(*pairs[j])
                for b in range(BPC):
                    emit_v(b)
                for i, (b, hp) in enumerate(pairs):
                    if i + DEPTH < len(pairs):
                        gmap[pairs[i + DEPTH]] = windows(*pairs[i + DEPTH])
                    phase_b(b, hp, gmap.pop((b, hp)))

    nc.finalize()
    return nc


def _get_program():
    if "nc" not in _CACHE:
        _CACHE["nc"] = _build_program()
    return _CACHE["nc"]


def prepare_in_maps(inputs):
    import ml_dtypes

    bf16 = ml_dtypes.bfloat16
    f32 = np.float32

    hs = np.asarray(inputs["hidden_states"], f32)
    am = np.asarray(inputs["attention_mask"], f32).reshape(B, S)
    cm = np.asarray(inputs["character_level_matrix"]).astype(np.int64)
    wm = np.asarray(inputs["word_level_matrix"]).astype(np.int64)

    Wq = np.asarray(inputs["Wq"], f32)
    Wk = np.asarray(inputs["Wk"], f32)
    Wv = np.asarray(inputs["Wv"], f32)
    bq = np.asarray(inputs["bq"], f32)
    bk = np.asarray(inputs["bk"], f32)
    bv = np.asarray(inputs["bv"], f32)
    E = np.asarray(inputs["dist_emb"], f32)
    chtab = np.asarray(inputs["char_emb"], f32)[:, 0]
    wdtab = np.asarray(inputs["word_emb"], f32)[:, 0]

    # x^T per batch: [B, HID, S] bf16
    xts = np.ascontiguousarray(hs.transpose(0, 2, 1)).astype(bf16)

    # char/word level bias, pre-sheared into k-window coordinates:
    # biasW[b, t, p, w] = bias[b, l=w+p-127, r=128t+p] / 16  (0 out of range)
    bias_full = ((chtab[cm] + wdtab[wm]) / 16.0).astype(f32)  # [B, S(l), S(r)]
    p_idx = np.arange(P)
    w_idx = np.arange(W)
    l_idx = w_idx[None, :] + p_idx[:, None] - (P - 1)          # [P, W]
    valid = (l_idx >= 0) & (l_idx < S)
    l_clip = np.clip(l_idx, 0, S - 1)
    biasW = np.zeros((B, NLT, P, W), f32)
    for t in range(NLT):
        r_idx = t * P + p_idx                                  # [P]
        # bias_full[b, l_clip, r] -> [B, P, W]
        biasW[:, t] = np.where(valid[None],
                               bias_full[:, l_clip, r_idx[:, None]], 0.0)
    biasW = biasW.astype(bf16)

    # weights packed [18, 128, 768] fp8, scaled 8x (evictions rescale)
    wall = np.concatenate([
        (8.0 * Wq).T.reshape(6, P, HID),
        (8.0 * Wk).T.reshape(6, P, HID),
        (8.0 * Wv).T.reshape(6, P, HID)], axis=0).astype(bf16)
    bqk = np.concatenate([(bq / 8.0).reshape(6, P), bk.reshape(6, P)],
                         axis=0).astype(f32)
    etr2 = np.vstack([E[::-1].T, E[::-1].T]).astype(bf16)       # [128, 1023]
    et82 = np.vstack([(E / 8.0).T, (E / 8.0).T]).astype(bf16)
    shared = {
        "wall": np.ascontiguousarray(wall),
        "bqk": np.ascontiguousarray(bqk),
        "bv2": bv.reshape(1, HID),
        "etr2": np.ascontiguousarray(etr2),
        "et82": np.ascontiguousarray(et82),
    }
    in_maps = []
    for c in range(8):
        sl = slice(c * BPC, (c + 1) * BPC)
        in_maps.append({
            "xt": xts[sl], "biasw": np.ascontiguousarray(biasW[sl]),
            "maskr": am[sl], **shared,
        })
    return in_maps


def kernel(**inputs):
    from concourse.bass_utils import run_bass_kernel_spmd

    in_maps = prepare_in_maps(inputs)
    nc = _get_program()
    res = run_bass_kernel_spmd(nc, in_maps, core_ids=list(range(8)))
    _CACHE["last_result"] = res
    # device output: [BPC, H, 65, S] bf16 ctxT (row 64 = softmax denominator)
    raw = np.concatenate([res.results[c]["out"] for c in range(8)], axis=0)
    raw = raw.astype(np.float32)
    ctx = raw[:, :, 0:64, :] / raw[:, :, 64:65, :]       # [B, H, D, S]
    out = ctx.transpose(0, 3, 1, 2).reshape(B, S, HID)
    return np.ascontiguousarray(out)

